# revision 23
# baseline (speedup 1.0000x reference)
"""GAT 3-layer Bass kernel for 8 trn2 cores.

v3: fp8 256B htab rows; per-seg gathers spread over 4 SWDGE queues
(Q7 core-pair concurrency); host-precomputed edge-major one-hot (no
on-chip is_equal); PSUM reads on the scalar engine; batched PSUM
evacuation; layer-1 gather eliminated via host-pregathered xgT; split
collectives.
"""
import numpy as np
import concourse.bacc as bacc
import concourse.bass as bass
from concourse import bass_utils
from concourse.tile import TileContext
import concourse.mybir as mybir

N, H, C_OUT, G = 50000, 128, 10, 128
NCORES = 8
NPC = N // NCORES            # 6250
WPC = 49                     # 128-node dst windows per core
CHUNK_W = 2
NCHUNK = (WPC + CHUNK_W - 1) // CHUNK_W   # 25
SHARD_PAD = WPC * 128        # 6272
NPAD = SHARD_PAD * NCORES    # 50176
ROW_B = 256                  # 256B row: [h fp8 x128 | as f32 | pad]
LO_ROWS = 32768
EXP_SHIFT = 4.0
NQ = 4                       # SWDGE queues (Q7 core pairs)
HALF_W = 25                  # windows in first collective half
HALF_CH = 12                 # after this chunk's evac, windows 0..25 are done

F16, F32, I16 = mybir.dt.float16, mybir.dt.float32, mybir.dt.int16
F8 = mybir.dt.float8e4
AF = mybir.ActivationFunctionType
OP = mybir.AluOpType


def prep_edges(edge_index):
    src = np.concatenate([edge_index[0], np.arange(N)]).astype(np.int64)
    dst = np.concatenate([edge_index[1], np.arange(N)]).astype(np.int64)
    row_id = (src // NPC) * SHARD_PAD + (src % NPC)

    per_core = []
    for c in range(NCORES):
        m = (dst // NPC) == c
        s_r, d_c = row_id[m], dst[m] - c * NPC
        win = d_c // 128
        core_chunks = []
        for ch in range(NCHUNK):
            wids = [w for w in (2 * ch, 2 * ch + 1) if w < WPC]
            segs = {}
            for hi in (0, 1):
                for w in wids:
                    mm = (win == w) & ((s_r >= LO_ROWS) == bool(hi))
                    rows = s_r[mm]
                    o = np.argsort(rows, kind="stable")
                    segs[(w, hi)] = (rows[o], (d_c[mm] - w * 128)[o])
            core_chunks.append(segs)
        per_core.append(core_chunks)

    chunks_meta = []
    for ch in range(NCHUNK):
        meta = []
        for key in per_core[0][ch]:
            w, hi = key
            mx = max(len(per_core[c][ch][key][0]) for c in range(NCORES))
            meta.append((w, hi, max(1, -(-mx // 128))))
        meta.sort(key=lambda x: (x[1], x[0]))  # lo segs first, then hi
        chunks_meta.append(meta)

    idx_lo = [[] for _ in range(NCORES)]
    idx_hi = [[] for _ in range(NCORES)]
    dstl = [[] for _ in range(NCORES)]
    rows_all = [[] for _ in range(NCORES)]
    for ch in range(NCHUNK):
        for (w, hi, ntile) in chunks_meta[ch]:
            L = ntile * 128
            for c in range(NCORES):
                rows, dl = per_core[c][ch][(w, hi)]
                r = np.zeros(L, np.int64)
                d = np.full(L, -1.0, np.float32)
                r[: len(rows)] = rows - (LO_ROWS if hi else 0)
                d[: len(dl)] = dl
                (idx_hi if hi else idx_lo)[c].append(r)
                dstl[c].append(d)
                rfull = np.zeros(L, np.int64)
                rfull[: len(rows)] = rows
                rows_all[c].append(rfull)

    def wrap16(a):
        a = a.astype(np.int16).reshape(-1, 16).T
        return np.tile(a, (8, 1)).copy()

    cores = []
    for c in range(NCORES):
        lo = np.concatenate(idx_lo[c]); hi = np.concatenate(idx_hi[c])
        dl = np.concatenate(dstl[c])
        dstl_pt = dl.reshape(-1, 128).T.copy()  # [128(edge), T]
        rows_c = np.concatenate(rows_all[c])
        shard = rows_c // SHARD_PAD
        local = rows_c % SHARD_PAD
        nodes = np.minimum(shard * NPC + local, N - 1)
        cores.append(dict(
            idxlo=wrap16(lo), idxhi=wrap16(hi),
            dstl=dstl_pt, src_nodes=nodes,
        ))
    T_total = sum(nt for ch in chunks_meta for (_, _, nt) in ch)
    n_lo = sum(nt * 128 for ch in chunks_meta for (_, hi, nt) in ch if not hi)
    n_hi = sum(nt * 128 for ch in chunks_meta for (_, hi, nt) in ch if hi)
    return chunks_meta, cores, T_total, n_lo, n_hi


def make_onehots(dstl_pt):
    """m0e[p, t*128+j] = 1 if dstl[p,t]==j (edge-major, scatter lhsT).
    m0t[n, t*128+j] = 1 if dstl[j,t]==n (dst-major, adx lhsT)."""
    T = dstl_pt.shape[1]
    ar = np.arange(128)
    m0e = (dstl_pt[:, :, None] == ar[None, None, :]).reshape(128, T * 128)
    dmat = dstl_pt.T.reshape(T, 128)
    m0t = (ar[:, None, None] == dmat[None, :, :]).reshape(128, T * 128)
    f8 = mybir.dt.np(F8)
    return m0e.astype(f8), m0t.astype(f8)


def make_weight_inputs(W1, a_src1, a_dst1, b1, W2, a_src2, a_dst2, b2,
                       W3, a_src3, a_dst3, b3, lin_W, lin_b):
    waug = np.zeros((128, 3, 130), np.float16)
    brep = np.zeros((128, 3, 128), np.float16)
    for i, (W, asr, ads, b) in enumerate([(W1, a_src1, a_dst1, b1),
                                          (W2, a_src2, a_dst2, b2),
                                          (W3, a_src3, a_dst3, b3)]):
        waug[:, i, 0:128] = W.astype(np.float32)
        waug[:, i, 128] = (W.astype(np.float64) @ asr.astype(np.float64)).astype(np.float32)
        waug[:, i, 129] = (W.astype(np.float64) @ ads.astype(np.float64)).astype(np.float32)
        brep[:, i, :] = np.broadcast_to(b.astype(np.float32), (128, 128))
    iota = np.broadcast_to(np.arange(128, dtype=np.float16), (128, 128)).copy()
    return dict(
        waug=waug, brep=brep,
        linw=lin_W.astype(np.float16),
        linb=np.broadcast_to(lin_b.astype(np.float32), (128, C_OUT)).copy(),
        iota=iota, idm=np.eye(128, dtype=np.float16),
    )


def make_xsT0(x, core):
    out = np.zeros((128, SHARD_PAD), np.float16)
    out[:, :NPC] = x[core * NPC:(core + 1) * NPC].astype(np.float16).T
    return out


def make_batch_input(batch, core):
    bl = np.full((128, WPC), -1.0, np.float32)
    ids = batch[core * NPC:(core + 1) * NPC].astype(np.float32)
    for w in range(WPC):
        seg = ids[w * 128:(w + 1) * 128]
        bl[: len(seg), w] = seg
    return bl


def make_ob_input(bl):
    """obt[p, w, j] = 1 if batch[node (w,p)] == j  (pool one-hot)."""
    return (bl[:, :, None] == np.arange(128)[None, None, :]).astype(np.float16)


def split_waits(nc, maxw=1):
    n = 0
    for func in nc.m.functions:
        for block in func.blocks:
            new = []
            for inst in block.instructions:
                si = inst.sync_info
                if si is not None and si.on_wait and len(si.on_wait) > maxw:
                    w = list(si.on_wait); extra, keep = w[:-maxw], w[-maxw:]
                    while extra:
                        ck, extra = extra[:maxw], extra[maxw:]
                        new.append(mybir.InstNoOp(name=f"ws-{n}", engine=inst.engine,
                            sync_info=mybir.SyncInfo(on_wait=ck, on_update=[])))
                        n += 1
                    si.on_wait = keep
                new.append(inst)
            block.instructions = new
    return n


def build(nc, chunks_meta, T_total, n_lo, n_hi, n_layers=3, with_pool=True,
          dump_xsT=False):
    waug_in = nc.dram_tensor("waug", [128, 3, 130], F16, kind="ExternalInput")
    brep_in = nc.dram_tensor("brep", [128, 3, 128], F16, kind="ExternalInput")
    linw_in = nc.dram_tensor("linw", [128, C_OUT], F16, kind="ExternalInput")
    linb_in = nc.dram_tensor("linb", [128, C_OUT], F32, kind="ExternalInput")
    iota_in = nc.dram_tensor("iota", [128, 128], F16, kind="ExternalInput")
    idm_in  = nc.dram_tensor("idm", [128, 128], F16, kind="ExternalInput")
    bl_in   = nc.dram_tensor("batchl", [128, WPC], F32, kind="ExternalInput")
    obt_in  = nc.dram_tensor("obt", [128, WPC, 128], F16, kind="ExternalInput")
    ilo_in  = nc.dram_tensor("idxlo", [128, n_lo // 16], I16, kind="ExternalInput")
    ihi_in  = nc.dram_tensor("idxhi", [128, n_hi // 16], I16, kind="ExternalInput")
    m0e_in  = nc.dram_tensor("m0e", [128, T_total * 128], F8, kind="ExternalInput")
    m0t_in  = nc.dram_tensor("m0t", [128, T_total * 128], F8, kind="ExternalInput")
    xsT0_in = nc.dram_tensor("xsT0", [128, SHARD_PAD], F16, kind="ExternalInput")
    xgT_in  = nc.dram_tensor("xgT", [128, T_total * 128], F16, kind="ExternalInput")
    out_t   = nc.dram_tensor("out", [G, C_OUT], F32, kind="ExternalOutput")
    xsT_out = nc.dram_tensor("xsT_out", [128, SHARD_PAD], F16, kind="ExternalOutput") if dump_xsT else None

    CT_MAX = max(sum(nt for (_, _, nt) in ch) for ch in chunks_meta)
    W_HALF0 = HALF_W * 128
    W_HALF1 = SHARD_PAD - W_HALF0

    with TileContext(nc) as tc:
        with tc.tile_pool(name="const", bufs=1) as constp, \
             tc.tile_pool(name="xTp", bufs=1) as xtp, \
             tc.tile_pool(name="gath", bufs=4) as gathp, \
             tc.tile_pool(name="gt16p", bufs=2) as gt16p, \
             tc.tile_pool(name="m0p", bufs=2) as m0p, \
             tc.tile_pool(name="ewp", bufs=2) as ewp, \
             tc.tile_pool(name="evac", bufs=3) as evp, \
             tc.tile_pool(name="stage", bufs=3) as stp, \
             tc.tile_pool(name="psw", bufs=3, space="PSUM") as psw, \
             tc.tile_pool(name="pst", bufs=3, space="PSUM") as pst, \
             tc.tile_pool(name="psadx", bufs=2, space="PSUM") as psadx, \
             tc.tile_pool(name="m0tp", bufs=2) as m0tp, \
             tc.tile_pool(name="dram", bufs=1, space="DRAM") as dram:

            xsT  = xtp.tile([128, SHARD_PAD], F16)
            waug = constp.tile([128, 3, 130], F16)
            brep = constp.tile([128, 3, 128], F16)
            linw = constp.tile([128, C_OUT], F16)
            linb = constp.tile([128, C_OUT], F32)
            iota = constp.tile([128, 128], F16)
            idm  = constp.tile([128, 128], F16)
            bl   = constp.tile([128, WPC], F32)
            obt  = constp.tile([128, WPC, 128], F16)
            ilo  = constp.tile([128, n_lo // 16], I16)
            ihi  = constp.tile([128, n_hi // 16], I16)
            nc.sync.dma_start(out=xsT[:], in_=xsT0_in[:])
            for t, s in [(waug, waug_in), (brep, brep_in),
                         (linw, linw_in), (linb, linb_in), (iota, iota_in),
                         (idm, idm_in), (bl, bl_in), (obt, obt_in),
                         (ilo, ilo_in), (ihi, ihi_in)]:
                nc.sync.dma_start(out=t[:], in_=s[:])

            negshift = constp.tile([128, 1], F32)
            nc.vector.memset(negshift[:], -EXP_SHIFT)
            eps = constp.tile([128, 1], F32)
            nc.vector.memset(eps[:], 1e-6)
            htabs = {lyr: dram.tile([NPAD, ROW_B], F8, name=f"htab_{lyr}", tag=f"htab_{lyr}")
                     for lyr in range(1, n_layers)}
            bi1 = {}; bo1 = {}; bi2 = {}; bo2 = {}
            for lyr in range(1, n_layers):
                bi1[lyr] = dram.tile([128, W_HALF0], F16, name=f"bi1_{lyr}", tag=f"bi1_{lyr}")
                bo1[lyr] = dram.tile([NCORES, 128, W_HALF0], F16, addr_space="Shared", name=f"bo1_{lyr}", tag=f"bo1_{lyr}")
                bi2[lyr] = dram.tile([128, W_HALF1], F16, name=f"bi2_{lyr}", tag=f"bi2_{lyr}")
                bo2[lyr] = dram.tile([NCORES, 128, W_HALF1], F16, addr_space="Shared", name=f"bo2_{lyr}", tag=f"bo2_{lyr}")
            pool_biA = dram.tile([128, 129], F32)
            pool_boA = dram.tile([128, 129], F32, addr_space="Shared")
            pool_biB = dram.tile([128, 129], F32)
            pool_boB = dram.tile([128, 129], F32, addr_space="Shared")

            # =========================================================
            B3 = 3

            def emit_table_batch(layer, bo, col0, r, w0, bn):
                """One 3-window batch of the fp8 htab rebuild for `layer`."""
                sti = stp.tile([128, B3 * 128], F16, tag="sti", name="sti")
                nc.sync.dma_start(
                    out=sti[:, 0:bn * 128],
                    in_=bo[r, :, w0 * 128:(w0 + bn) * 128])
                ps3 = pst.tile([128, 390], F32, tag="tab", name="tabps")
                for j in range(bn):
                    nc.tensor.matmul(ps3[:, j * 130:j * 130 + 129],
                                     sti[:, j * 128:(j + 1) * 128],
                                     waug[:, layer, 0:129], start=True, stop=True,
                                     skip_group_check=True)
                stg = stp.tile([128, B3, ROW_B], F8, tag="stg", name="stg")
                stg32 = stg[:].bitcast(F32)
                ps3v = ps3[:].rearrange("p (t e) -> p t e", e=130)
                if w0 % 2 == 0:
                    nc.vector.tensor_copy(stg[:, 0:bn, 0:128], ps3v[:, 0:bn, 0:128])
                else:
                    nc.scalar.activation(stg[:, 0:bn, 0:128], ps3v[:, 0:bn, 0:128], AF.Copy)
                nc.vector.tensor_copy(stg32[:, 0:bn, 32:33], ps3v[:, 0:bn, 128:129])
                row0 = r * SHARD_PAD + (col0 + w0) * 128
                nc.sync.dma_start(
                    out=htabs[layer][row0:row0 + bn * 128, :]
                        .rearrange("(b p) e -> p b e", p=128),
                    in_=stg[:, 0:bn, :])

            def table_gen(layer):
                """Half-1 batches (windows 0..HALF_W-1) — interleavable."""
                for r in range(NCORES):
                    for w0 in range(0, HALF_W, B3):
                        yield (layer, bo1[layer], 0, r, w0, min(B3, HALF_W - w0))

            def table_rest(layer):
                """Half-2 batches (windows HALF_W..WPC-1)."""
                nwin = WPC - HALF_W
                for r in range(NCORES):
                    for w0 in range(0, nwin, B3):
                        emit_table_batch(layer, bo2[layer], HALF_W, r, w0,
                                         min(B3, nwin - w0))

            # =========================================================
            def edge_phase(layer, next_gen=None):
                last = (layer == n_layers - 1)
                feeds_next = (layer + 1 < n_layers)
                adlps = pst.tile([128, WPC], F32, tag="tab", name=f"adlps_{layer}")
                for w in range(WPC):
                    nc.tensor.matmul(adlps[:, w:w + 1], xsT[:, w * 128:(w + 1) * 128],
                                     waug[:, layer, 129:130], start=True, stop=True,
                                     skip_group_check=True)
                adl16 = ewp.tile([128, WPC], F16, tag="adl16", name=f"adl16_{layer}", bufs=1)
                nc.scalar.activation(adl16[:], adlps[:], AF.Copy)
                pool_A = pst.tile([128, 129], F32, tag="tab", name="pool_A") if (with_pool and last) else None
                pool_B = pst.tile([128, 129], F32, tag="tab", name="pool_B") if (with_pool and last) else None
                pooledA = evp.tile([128, 129], F32, tag="pooled", name="pooledA") if (with_pool and last) else None

                t0 = 0; off_lo = 0; off_hi = 0
                for ch, meta in enumerate(chunks_meta):
                    ct = sum(nt for (_, _, nt) in meta)
                    ftw = {}
                    _tt = 0
                    for (w, hi, nt) in meta:
                        for _ in range(nt):
                            ftw[_tt] = w
                            _tt += 1

                    gt16 = gt16p.tile([128, CT_MAX, 130], F16, tag="g16")
                    as32 = ewp.tile([128, CT_MAX], F32, tag="as32", name="as32") if layer == 0 else None
                    gt8 = None
                    if layer == 0:
                        xgt = gathp.tile([128, CT_MAX, 128], F16, tag="g", name="xgt")
                        nc.sync.dma_start(out=xgt[:, 0:ct, :],
                                          in_=xgT_in[:, t0 * 128:(t0 + ct) * 128]
                                              .rearrange("p (t e) -> p t e", e=128))
                        for g3 in range(0, ct, 3):
                            bn = min(3, ct - g3)
                            ps3 = pst.tile([128, 390], F32, tag="tab", name="l0ps")
                            for j in range(bn):
                                nc.tensor.matmul(ps3[:, j * 130:j * 130 + 129],
                                                 xgt[:, g3 + j, :],
                                                 waug[:, 0, 0:129], start=True, stop=True,
                                                 skip_group_check=True)
                            ps3v = ps3[:].rearrange("p (t e) -> p t e", e=130)
                            nc.scalar.activation(gt16[:, g3:g3 + bn, 0:128],
                                                 ps3v[:, 0:bn, 0:128], AF.Copy)
                            nc.vector.tensor_copy(as32[:, g3:g3 + bn].unsqueeze(2),
                                                  ps3v[:, 0:bn, 128:129])
                    else:
                        gt8 = gathp.tile([128, CT_MAX, ROW_B], F8, tag="g", name="gt8")
                        htab = htabs[layer]
                        tt = 0
                        si = 0
                        for (w, want_hi, nt) in meta:
                            n_seg = nt * 128
                            src_ap = htab[LO_ROWS:NPAD, :] if want_hi else htab[0:LO_ROWS, :]
                            if want_hi:
                                idxs = ihi[:, off_hi // 16:(off_hi + n_seg) // 16]
                                off_hi += n_seg
                            else:
                                idxs = ilo[:, off_lo // 16:(off_lo + n_seg) // 16]
                                off_lo += n_seg
                            nc.gpsimd.dma_gather(
                                out_ap=gt8[:, tt:tt + nt, :], in_ap=src_ap,
                                idxs_ap=idxs, num_idxs=n_seg, num_idxs_reg=n_seg,
                                elem_size=ROW_B, single_packet=False,
                                queue_num=si % NQ)
                            tt += nt
                            si += 1

                    m0t = m0tp.tile([128, CT_MAX * 128], F8, tag="m0t")
                    nc.sync.dma_start(out=m0t[:, 0:ct * 128],
                                      in_=m0t_in[:, t0 * 128:(t0 + ct) * 128])
                    m0e = m0p.tile([128, CT_MAX * 128], F8, tag="m0")
                    nc.sync.dma_start(out=m0e[:, 0:ct * 128],
                                      in_=m0e_in[:, t0 * 128:(t0 + ct) * 128])
                    adx = psadx.tile([128, 512], F32, tag="adx", name=f"adx_{layer}_{ch}")
                    for _tt in range(ct):
                        nc.tensor.matmul(adx[:, _tt:_tt + 1],
                                         m0t[:, _tt * 128:(_tt + 1) * 128],
                                         adl16[:, ftw[_tt]:ftw[_tt] + 1],
                                         start=True, stop=True, skip_group_check=True)
                    adxs = ewp.tile([128, CT_MAX], F32, tag="adxs", name="adxs")
                    nc.scalar.activation(adxs[:, 0:ct], adx[:, 0:ct], AF.Copy)

                    z  = ewp.tile([128, CT_MAX], F32, tag="z")
                    e1 = ewp.tile([128, CT_MAX], F32, tag="e1")
                    ef = ewp.tile([128, CT_MAX], F32, tag="ef")
                    if layer == 0:
                        nc.vector.tensor_tensor(z[:, 0:ct], as32[:, 0:ct], adxs[:, 0:ct], OP.add)
                    else:
                        g32 = gt8[:].bitcast(F32)
                        nc.vector.tensor_tensor(z[:, 0:ct].unsqueeze(2),
                                                g32[:, 0:ct, 32:33],
                                                adxs[:, 0:ct].unsqueeze(2), OP.add)
                    nc.scalar.activation(e1[:, 0:ct], z[:, 0:ct], AF.Exp, bias=negshift[:])
                    nc.scalar.activation(z[:, 0:ct], z[:, 0:ct], AF.Exp, bias=negshift[:], scale=0.2)
                    nc.vector.tensor_tensor(ef[:, 0:ct], e1[:, 0:ct], z[:, 0:ct], OP.max)

                    # weighted rows: gt16[:, :, 0:128] = h * ef, col 128 = ef
                    # (alternate DVE / ACT per tile to split the load)
                    for _tt in range(ct):
                        src = gt16 if layer == 0 else gt8
                        if _tt % 2 == 0:
                            nc.vector.tensor_scalar_mul(gt16[:, _tt, 0:128],
                                                        src[:, _tt, 0:128],
                                                        ef[:, _tt:_tt + 1])
                        else:
                            nc.scalar.activation(gt16[:, _tt, 0:128],
                                                 src[:, _tt, 0:128], AF.Copy,
                                                 scale=ef[:, _tt:_tt + 1])
                    nc.scalar.activation(gt16[:, 0:ct, 128:129],
                                         ef[:, 0:ct].unsqueeze(2), AF.Copy)

                    ft, lt = {}, {}
                    tt = 0
                    for (w, hi, nt) in meta:
                        for _ in range(nt):
                            if w not in ft: ft[w] = tt
                            lt[w] = tt
                            tt += 1
                    psws = {w: psw.tile([128, 129], F32, tag="win", name=f"win_{layer}_{ch}_{w}") for w in ft}
                    tt = 0
                    for (w, hi, nt) in meta:
                        for _ in range(nt):
                            nc.tensor.matmul(psws[w][:], m0e[:, tt * 128:(tt + 1) * 128],
                                             gt16[:, tt, 0:129],
                                             start=(tt == ft[w]), stop=(tt == lt[w]),
                                             skip_group_check=True)
                            tt += 1
                    for w in sorted(ft):
                        ps = psws[w]
                        dn = evp.tile([128, 1], F32, tag="dn")
                        nc.scalar.activation(dn[:], ps[:, 128:129], AF.Copy, bias=1e-6)
                        rc = evp.tile([128, 1], F32, tag="rc")
                        nc.vector.reciprocal(rc[:], dn[:])
                        xw = evp.tile([128, 129], F16, tag="xw")
                        nc.scalar.activation(xw[:, 0:128], ps[:, 0:128], AF.Copy, scale=rc[:])
                        nc.vector.tensor_tensor(xw[:, 0:128], xw[:, 0:128], brep[:, layer, :], OP.add)
                        nc.scalar.activation(xw[:, 0:128], xw[:, 0:128], AF.Relu)
                        if not (with_pool and last):
                            tp = psw.tile([128, 128], F16, tag="win", name=f"tp_{layer}_{ch}_{w}")
                            nc.tensor.transpose(tp[:], xw[:, 0:128], idm[:])
                            nc.vector.tensor_copy(xsT[:, w * 128:(w + 1) * 128], tp[:])
                        else:
                            nc.vector.memset(xw[:, 128:129], 1.0)
                            pps = pool_A if w < HALF_W else pool_B
                            nc.tensor.matmul(pps[:], obt[:, w, :], xw[:, 0:129],
                                             start=(w in (0, HALF_W)),
                                             stop=(w in (HALF_W - 1, WPC - 1)),
                                             skip_group_check=True)

                    if next_gen is not None and ch >= 17:
                        for _ in range(9):
                            args = next(next_gen, None)
                            if args is None:
                                break
                            emit_table_batch(*args)
                    if ch == HALF_CH:
                        if feeds_next:
                            nc.sync.dma_start(out=bi1[layer + 1][:], in_=xsT[:, 0:W_HALF0])
                            nc.gpsimd.collective_compute(
                                "AllGather", OP.bypass, replica_groups=[list(range(NCORES))],
                                ins=[bi1[layer + 1][:].opt()], outs=[bo1[layer + 1][:].opt()])
                        if with_pool and last:
                            nc.scalar.activation(pooledA[:], pool_A[:], AF.Copy)
                            nc.sync.dma_start(out=pool_biA[:], in_=pooledA[:])
                            nc.gpsimd.collective_compute(
                                "AllReduce", OP.add, replica_groups=[list(range(NCORES))],
                                ins=[pool_biA[:].opt()], outs=[pool_boA[:].opt()])
                    t0 += ct

                if feeds_next:
                    nc.sync.dma_start(out=bi2[layer + 1][:], in_=xsT[:, W_HALF0:SHARD_PAD])
                    nc.gpsimd.collective_compute(
                        "AllGather", OP.bypass, replica_groups=[list(range(NCORES))],
                        ins=[bi2[layer + 1][:].opt()], outs=[bo2[layer + 1][:].opt()])
                return pool_B

            # ================= main =================
            pool_B = None
            for layer in range(n_layers):
                gen = table_gen(layer + 1) if layer + 1 < n_layers else None
                pb = edge_phase(layer, next_gen=gen)
                if pb is not None:
                    pool_B = pb
                if gen is not None:
                    for args in gen:
                        emit_table_batch(*args)
                if layer + 1 < n_layers:
                    table_rest(layer + 1)

            if dump_xsT:
                nc.sync.dma_start(out=xsT_out[:], in_=xsT[:])
            if not with_pool:
                zz = evp.tile([128, C_OUT], F32, tag="res")
                nc.vector.memset(zz[:], 0.0)
                nc.sync.dma_start(out=out_t[:], in_=zz[:])
                return nc

            pooledB = evp.tile([128, 129], F32, tag="pooled", name="pooledB")
            nc.scalar.activation(pooledB[:], pool_B[:], AF.Copy)
            nc.sync.dma_start(out=pool_biB[:], in_=pooledB[:])
            nc.gpsimd.collective_compute(
                "AllReduce", OP.add, replica_groups=[list(range(NCORES))],
                ins=[pool_biB[:].opt()], outs=[pool_boB[:].opt()])
            pA = evp.tile([128, 129], F32, tag="pooled", name="pA")
            pB = evp.tile([128, 129], F32, tag="pooled", name="pB")
            nc.sync.dma_start(out=pA[:], in_=pool_boA[:])
            nc.sync.dma_start(out=pB[:], in_=pool_boB[:])
            pooled = evp.tile([128, 129], F32, tag="pooled", name="pooled")
            nc.vector.tensor_tensor(pooled[:], pA[:], pB[:], OP.add)
            cnt = evp.tile([128, 1], F32, tag="cnt")
            nc.vector.tensor_scalar_max(cnt[:], pooled[:, 128:129], 1.0)
            rcn = evp.tile([128, 1], F32, tag="rcn")
            nc.vector.reciprocal(rcn[:], cnt[:])
            pm = evp.tile([128, 128], F16, tag="pm")
            nc.scalar.activation(pm[:], pooled[:, 0:128], AF.Copy, scale=rcn[:])
            pt = psw.tile([128, 128], F16, tag="win", name="pt_fin")
            nc.tensor.transpose(pt[:], pm[:], idm[:])
            pts = evp.tile([128, 128], F16, tag="pts")
            nc.vector.tensor_copy(pts[:], pt[:])
            ho = psw.tile([128, 129], F32, tag="win", name="ho_fin")
            nc.tensor.matmul(ho[:, 0:C_OUT], pts[:], linw[:], start=True, stop=True,
                             skip_group_check=True)
            res = evp.tile([128, C_OUT], F32, tag="res")
            nc.vector.tensor_tensor(res[:], ho[:, 0:C_OUT], linb[:], OP.add)
            nc.sync.dma_start(out=out_t[:], in_=res[:])
    return nc


def run(inputs, trace=False, n_layers=3, with_pool=True, dump_xsT=False):
    x = np.asarray(inputs["x"])
    chunks_meta, cores, T_total, n_lo, n_hi = prep_edges(np.asarray(inputs["edge_index"]))
    const_ins = make_weight_inputs(
        np.asarray(inputs["W1"]), np.asarray(inputs["a_src1"]), np.asarray(inputs["a_dst1"]), np.asarray(inputs["b1"]),
        np.asarray(inputs["W2"]), np.asarray(inputs["a_src2"]), np.asarray(inputs["a_dst2"]), np.asarray(inputs["b2"]),
        np.asarray(inputs["W3"]), np.asarray(inputs["a_src3"]), np.asarray(inputs["a_dst3"]), np.asarray(inputs["b3"]),
        np.asarray(inputs["lin_W"]), np.asarray(inputs["lin_b"]))
    batch = np.asarray(inputs["batch"])

    nc = bacc.Bacc("TRN2", target_bir_lowering=False, debug=False,
                   num_devices=NCORES, num_swdge_queues=NQ)
    build(nc, chunks_meta, T_total, n_lo, n_hi, n_layers=n_layers,
          with_pool=with_pool, dump_xsT=dump_xsT)
    nc.compile()
    split_waits(nc)

    xf16 = x.astype(np.float16)
    in_maps = []
    for c in range(NCORES):
        m = dict(const_ins)
        m["batchl"] = make_batch_input(batch, c)
        m["obt"] = make_ob_input(m["batchl"])
        m["idxlo"] = cores[c]["idxlo"]
        m["idxhi"] = cores[c]["idxhi"]
        m0e, m0t = make_onehots(cores[c]["dstl"])
        m["m0e"] = m0e
        m["m0t"] = m0t
        m["xsT0"] = make_xsT0(x, c)
        m["xgT"] = np.ascontiguousarray(xf16[cores[c]["src_nodes"]].T)
        in_maps.append(m)
    res = bass_utils.run_bass_kernel_spmd(nc, in_maps, core_ids=list(range(NCORES)),
                                          trace=trace)
    return res.results[0], res


def kernel(**inputs):
    """Harness entry: full unsharded inputs -> [128, 10] fp32 output."""
    out, _ = run(inputs)
    if isinstance(out, dict):
        out = out["out"]
    return np.asarray(out, dtype=np.float32)


# revision 24
# speedup vs baseline: 1.0793x; 1.0793x over previous
"""GAT 3-layer Bass kernel for 8 trn2 cores.

v3: fp8 256B htab rows; per-seg gathers spread over 4 SWDGE queues
(Q7 core-pair concurrency); host-precomputed edge-major one-hot (no
on-chip is_equal); PSUM reads on the scalar engine; batched PSUM
evacuation; layer-1 gather eliminated via host-pregathered xgT; split
collectives.
"""
import numpy as np
import concourse.bacc as bacc
import concourse.bass as bass
from concourse import bass_utils
from concourse.tile import TileContext
import concourse.mybir as mybir

N, H, C_OUT, G = 50000, 128, 10, 128
NCORES = 8
NPC = N // NCORES            # 6250
WPC = 49                     # 128-node dst windows per core
CHUNK_W = 2
NCHUNK = (WPC + CHUNK_W - 1) // CHUNK_W   # 25
SHARD_PAD = WPC * 128        # 6272
NPAD = SHARD_PAD * NCORES    # 50176
ROW_B = 256                  # 256B row: [h fp8 x128 | as f32 | pad]
LO_ROWS = 32768
EXP_SHIFT = 4.0
NQ = 4                       # SWDGE queues (Q7 core pairs)
HALF_W = 25                  # windows in first collective half
HALF_CH = 12                 # after this chunk's evac, windows 0..25 are done

F16, F32, I16 = mybir.dt.float16, mybir.dt.float32, mybir.dt.int16
F8 = mybir.dt.float8e4
AF = mybir.ActivationFunctionType
OP = mybir.AluOpType


def prep_edges(edge_index):
    src = np.concatenate([edge_index[0], np.arange(N)]).astype(np.int64)
    dst = np.concatenate([edge_index[1], np.arange(N)]).astype(np.int64)
    row_id = (src // NPC) * SHARD_PAD + (src % NPC)

    per_core = []
    for c in range(NCORES):
        m = (dst // NPC) == c
        s_r, d_c = row_id[m], dst[m] - c * NPC
        win = d_c // 128
        core_chunks = []
        for ch in range(NCHUNK):
            wids = [w for w in (2 * ch, 2 * ch + 1) if w < WPC]
            segs = {}
            for hi in (0, 1):
                for w in wids:
                    mm = (win == w) & ((s_r >= LO_ROWS) == bool(hi))
                    rows = s_r[mm]
                    o = np.argsort(rows, kind="stable")
                    segs[(w, hi)] = (rows[o], (d_c[mm] - w * 128)[o])
            core_chunks.append(segs)
        per_core.append(core_chunks)

    chunks_meta = []
    for ch in range(NCHUNK):
        meta = []
        for key in per_core[0][ch]:
            w, hi = key
            mx = max(len(per_core[c][ch][key][0]) for c in range(NCORES))
            meta.append((w, hi, max(1, -(-mx // 128))))
        meta.sort(key=lambda x: (x[1], x[0]))  # lo segs first, then hi
        chunks_meta.append(meta)

    idx_lo = [[] for _ in range(NCORES)]
    idx_hi = [[] for _ in range(NCORES)]
    dstl = [[] for _ in range(NCORES)]
    rows_all = [[] for _ in range(NCORES)]
    for ch in range(NCHUNK):
        for (w, hi, ntile) in chunks_meta[ch]:
            L = ntile * 128
            for c in range(NCORES):
                rows, dl = per_core[c][ch][(w, hi)]
                r = np.zeros(L, np.int64)
                d = np.full(L, -1.0, np.float32)
                r[: len(rows)] = rows - (LO_ROWS if hi else 0)
                d[: len(dl)] = dl
                (idx_hi if hi else idx_lo)[c].append(r)
                dstl[c].append(d)
                rfull = np.zeros(L, np.int64)
                rfull[: len(rows)] = rows
                rows_all[c].append(rfull)

    def wrap16(a):
        a = a.astype(np.int16).reshape(-1, 16).T
        return np.tile(a, (8, 1)).copy()

    cores = []
    for c in range(NCORES):
        lo = np.concatenate(idx_lo[c]); hi = np.concatenate(idx_hi[c])
        dl = np.concatenate(dstl[c])
        dstl_pt = dl.reshape(-1, 128).T.copy()  # [128(edge), T]
        rows_c = np.concatenate(rows_all[c])
        shard = rows_c // SHARD_PAD
        local = rows_c % SHARD_PAD
        nodes = np.minimum(shard * NPC + local, N - 1)
        cores.append(dict(
            idxlo=wrap16(lo), idxhi=wrap16(hi),
            dstl=dstl_pt, src_nodes=nodes,
        ))
    T_total = sum(nt for ch in chunks_meta for (_, _, nt) in ch)
    n_lo = sum(nt * 128 for ch in chunks_meta for (_, hi, nt) in ch if not hi)
    n_hi = sum(nt * 128 for ch in chunks_meta for (_, hi, nt) in ch if hi)
    return chunks_meta, cores, T_total, n_lo, n_hi


def make_onehots(dstl_pt):
    """m0e[p, t*128+j] = 1 if dstl[p,t]==j (edge-major, scatter lhsT).
    m0t[n, t*128+j] = 1 if dstl[j,t]==n (dst-major, adx lhsT)."""
    T = dstl_pt.shape[1]
    ar = np.arange(128)
    m0e = (dstl_pt[:, :, None] == ar[None, None, :]).reshape(128, T * 128)
    dmat = dstl_pt.T.reshape(T, 128)
    m0t = (ar[:, None, None] == dmat[None, :, :]).reshape(128, T * 128)
    f8 = mybir.dt.np(F8)
    return m0e.astype(f8), m0t.astype(f8)


def make_weight_inputs(W1, a_src1, a_dst1, b1, W2, a_src2, a_dst2, b2,
                       W3, a_src3, a_dst3, b3, lin_W, lin_b):
    waug = np.zeros((128, 3, 130), np.float16)
    brep = np.zeros((128, 3, 128), np.float16)
    for i, (W, asr, ads, b) in enumerate([(W1, a_src1, a_dst1, b1),
                                          (W2, a_src2, a_dst2, b2),
                                          (W3, a_src3, a_dst3, b3)]):
        waug[:, i, 0:128] = W.astype(np.float32)
        waug[:, i, 128] = (W.astype(np.float64) @ asr.astype(np.float64)).astype(np.float32)
        waug[:, i, 129] = (W.astype(np.float64) @ ads.astype(np.float64)).astype(np.float32)
        brep[:, i, :] = np.broadcast_to(b.astype(np.float32), (128, 128))
    iota = np.broadcast_to(np.arange(128, dtype=np.float16), (128, 128)).copy()
    return dict(
        waug=waug, brep=brep,
        linw=lin_W.astype(np.float16),
        linb=np.broadcast_to(lin_b.astype(np.float32), (128, C_OUT)).copy(),
        iota=iota, idm=np.eye(128, dtype=np.float16),
    )


def make_xsT0(x, core):
    out = np.zeros((128, SHARD_PAD), np.float16)
    out[:, :NPC] = x[core * NPC:(core + 1) * NPC].astype(np.float16).T
    return out


def make_batch_input(batch, core):
    bl = np.full((128, WPC), -1.0, np.float32)
    ids = batch[core * NPC:(core + 1) * NPC].astype(np.float32)
    for w in range(WPC):
        seg = ids[w * 128:(w + 1) * 128]
        bl[: len(seg), w] = seg
    return bl


def make_ob_input(bl):
    """obt[p, w, j] = 1 if batch[node (w,p)] == j  (pool one-hot)."""
    return (bl[:, :, None] == np.arange(128)[None, None, :]).astype(np.float16)


def split_waits(nc, maxw=1):
    n = 0
    for func in nc.m.functions:
        for block in func.blocks:
            new = []
            for inst in block.instructions:
                si = inst.sync_info
                if si is not None and si.on_wait and len(si.on_wait) > maxw:
                    w = list(si.on_wait); extra, keep = w[:-maxw], w[-maxw:]
                    while extra:
                        ck, extra = extra[:maxw], extra[maxw:]
                        new.append(mybir.InstNoOp(name=f"ws-{n}", engine=inst.engine,
                            sync_info=mybir.SyncInfo(on_wait=ck, on_update=[])))
                        n += 1
                    si.on_wait = keep
                new.append(inst)
            block.instructions = new
    return n


def build(nc, chunks_meta, T_total, n_lo, n_hi, n_layers=3, with_pool=True,
          dump_xsT=False):
    waug_in = nc.dram_tensor("waug", [128, 3, 130], F16, kind="ExternalInput")
    brep_in = nc.dram_tensor("brep", [128, 3, 128], F16, kind="ExternalInput")
    linw_in = nc.dram_tensor("linw", [128, C_OUT], F16, kind="ExternalInput")
    linb_in = nc.dram_tensor("linb", [128, C_OUT], F32, kind="ExternalInput")
    iota_in = nc.dram_tensor("iota", [128, 128], F16, kind="ExternalInput")
    idm_in  = nc.dram_tensor("idm", [128, 128], F16, kind="ExternalInput")
    bl_in   = nc.dram_tensor("batchl", [128, WPC], F32, kind="ExternalInput")
    obt_in  = nc.dram_tensor("obt", [128, WPC, 128], F16, kind="ExternalInput")
    ilo_in  = nc.dram_tensor("idxlo", [128, n_lo // 16], I16, kind="ExternalInput")
    ihi_in  = nc.dram_tensor("idxhi", [128, n_hi // 16], I16, kind="ExternalInput")
    m0e_in  = nc.dram_tensor("m0e", [128, T_total * 128], F8, kind="ExternalInput")
    m0t_in  = nc.dram_tensor("m0t", [128, T_total * 128], F8, kind="ExternalInput")
    xsT0_in = nc.dram_tensor("xsT0", [128, SHARD_PAD], F16, kind="ExternalInput")
    xgT_in  = nc.dram_tensor("xgT", [128, T_total * 128], F16, kind="ExternalInput")
    out_t   = nc.dram_tensor("out", [G, C_OUT], F32, kind="ExternalOutput")
    xsT_out = nc.dram_tensor("xsT_out", [128, SHARD_PAD], F16, kind="ExternalOutput") if dump_xsT else None

    CT_MAX = max(sum(nt for (_, _, nt) in ch) for ch in chunks_meta)
    W_HALF0 = HALF_W * 128
    W_HALF1 = SHARD_PAD - W_HALF0

    with TileContext(nc) as tc:
        with tc.tile_pool(name="const", bufs=1) as constp, \
             tc.tile_pool(name="xTp", bufs=1) as xtp, \
             tc.tile_pool(name="gath", bufs=4) as gathp, \
             tc.tile_pool(name="gt16p", bufs=2) as gt16p, \
             tc.tile_pool(name="m0p", bufs=2) as m0p, \
             tc.tile_pool(name="ewp", bufs=2) as ewp, \
             tc.tile_pool(name="evac", bufs=3) as evp, \
             tc.tile_pool(name="stage", bufs=3) as stp, \
             tc.tile_pool(name="psw", bufs=3, space="PSUM") as psw, \
             tc.tile_pool(name="pst", bufs=3, space="PSUM") as pst, \
             tc.tile_pool(name="psadx", bufs=2, space="PSUM") as psadx, \
             tc.tile_pool(name="m0tp", bufs=2) as m0tp, \
             tc.tile_pool(name="dram", bufs=1, space="DRAM") as dram:

            xsT  = xtp.tile([128, SHARD_PAD], F16)
            waug = constp.tile([128, 3, 130], F16)
            brep = constp.tile([128, 3, 128], F16)
            linw = constp.tile([128, C_OUT], F16)
            linb = constp.tile([128, C_OUT], F32)
            iota = constp.tile([128, 128], F16)
            idm  = constp.tile([128, 128], F16)
            bl   = constp.tile([128, WPC], F32)
            obt  = constp.tile([128, WPC, 128], F16)
            ilo  = constp.tile([128, n_lo // 16], I16)
            ihi  = constp.tile([128, n_hi // 16], I16)
            nc.sync.dma_start(out=xsT[:], in_=xsT0_in[:])
            for t, s in [(waug, waug_in), (brep, brep_in),
                         (linw, linw_in), (linb, linb_in), (iota, iota_in),
                         (idm, idm_in), (bl, bl_in), (obt, obt_in),
                         (ilo, ilo_in), (ihi, ihi_in)]:
                nc.sync.dma_start(out=t[:], in_=s[:])

            negshift = constp.tile([128, 1], F32)
            nc.vector.memset(negshift[:], -EXP_SHIFT)
            eps = constp.tile([128, 1], F32)
            nc.vector.memset(eps[:], 1e-6)
            htabs = {lyr: dram.tile([NPAD, ROW_B], F8, name=f"htab_{lyr}", tag=f"htab_{lyr}")
                     for lyr in range(1, n_layers)}
            bi1 = {}; bo1 = {}; bi2 = {}; bo2 = {}
            for lyr in range(1, n_layers):
                bi1[lyr] = dram.tile([128, W_HALF0], F16, name=f"bi1_{lyr}", tag=f"bi1_{lyr}")
                bo1[lyr] = dram.tile([NCORES, 128, W_HALF0], F16, addr_space="Shared", name=f"bo1_{lyr}", tag=f"bo1_{lyr}")
                bi2[lyr] = dram.tile([128, W_HALF1], F16, name=f"bi2_{lyr}", tag=f"bi2_{lyr}")
                bo2[lyr] = dram.tile([NCORES, 128, W_HALF1], F16, addr_space="Shared", name=f"bo2_{lyr}", tag=f"bo2_{lyr}")
            pool_biA = dram.tile([128, 129], F32)
            pool_boA = dram.tile([128, 129], F32, addr_space="Shared")
            pool_biB = dram.tile([128, 129], F32)
            pool_boB = dram.tile([128, 129], F32, addr_space="Shared")

            # =========================================================
            B3 = 3

            def emit_table_batch(layer, bo, col0, r, w0, bn):
                """One 3-window batch of the fp8 htab rebuild for `layer`."""
                sti = stp.tile([128, B3 * 128], F16, tag="sti", name="sti")
                nc.sync.dma_start(
                    out=sti[:, 0:bn * 128],
                    in_=bo[r, :, w0 * 128:(w0 + bn) * 128])
                ps3 = pst.tile([128, 390], F32, tag="tab", name="tabps")
                for j in range(bn):
                    nc.tensor.matmul(ps3[:, j * 130:j * 130 + 129],
                                     sti[:, j * 128:(j + 1) * 128],
                                     waug[:, layer, 0:129], start=True, stop=True,
                                     skip_group_check=True)
                stg = stp.tile([128, B3, ROW_B], F8, tag="stg", name="stg")
                stg32 = stg[:].bitcast(F32)
                ps3v = ps3[:].rearrange("p (t e) -> p t e", e=130)
                if w0 % 2 == 0:
                    nc.vector.tensor_copy(stg[:, 0:bn, 0:128], ps3v[:, 0:bn, 0:128])
                else:
                    nc.scalar.activation(stg[:, 0:bn, 0:128], ps3v[:, 0:bn, 0:128], AF.Copy)
                nc.vector.tensor_copy(stg32[:, 0:bn, 32:33], ps3v[:, 0:bn, 128:129])
                row0 = r * SHARD_PAD + (col0 + w0) * 128
                nc.sync.dma_start(
                    out=htabs[layer][row0:row0 + bn * 128, :]
                        .rearrange("(b p) e -> p b e", p=128),
                    in_=stg[:, 0:bn, :])

            def table_gen(layer):
                """Half-1 batches (windows 0..HALF_W-1) — interleavable."""
                for r in range(NCORES):
                    for w0 in range(0, HALF_W, B3):
                        yield (layer, bo1[layer], 0, r, w0, min(B3, HALF_W - w0))

            def table_rest(layer):
                """Half-2 batches (windows HALF_W..WPC-1)."""
                nwin = WPC - HALF_W
                for r in range(NCORES):
                    for w0 in range(0, nwin, B3):
                        emit_table_batch(layer, bo2[layer], HALF_W, r, w0,
                                         min(B3, nwin - w0))

            # =========================================================
            def edge_phase(layer, next_gen=None):
                last = (layer == n_layers - 1)
                feeds_next = (layer + 1 < n_layers)
                adlps = pst.tile([128, WPC], F32, tag="tab", name=f"adlps_{layer}")
                for w in range(WPC):
                    nc.tensor.matmul(adlps[:, w:w + 1], xsT[:, w * 128:(w + 1) * 128],
                                     waug[:, layer, 129:130], start=True, stop=True,
                                     skip_group_check=True)
                adl16 = ewp.tile([128, WPC], F16, tag="adl16", name=f"adl16_{layer}", bufs=1)
                nc.scalar.activation(adl16[:], adlps[:], AF.Copy)
                pool_A = pst.tile([128, 129], F32, tag="tab", name="pool_A") if (with_pool and last) else None
                pool_B = pst.tile([128, 129], F32, tag="tab", name="pool_B") if (with_pool and last) else None
                pooledA = evp.tile([128, 129], F32, tag="pooled", name="pooledA") if (with_pool and last) else None

                t0 = 0; off_lo = 0; off_hi = 0
                for ch, meta in enumerate(chunks_meta):
                    ct = sum(nt for (_, _, nt) in meta)
                    ftw = {}
                    _tt = 0
                    for (w, hi, nt) in meta:
                        for _ in range(nt):
                            ftw[_tt] = w
                            _tt += 1

                    gt16 = gt16p.tile([128, CT_MAX, 130], F16, tag="g16")
                    as32 = ewp.tile([128, CT_MAX], F32, tag="as32", name="as32") if layer == 0 else None
                    gt8 = None
                    if layer == 0:
                        xgt = gathp.tile([128, CT_MAX, 128], F16, tag="g", name="xgt")
                        nc.sync.dma_start(out=xgt[:, 0:ct, :],
                                          in_=xgT_in[:, t0 * 128:(t0 + ct) * 128]
                                              .rearrange("p (t e) -> p t e", e=128))
                        for g3 in range(0, ct, 3):
                            bn = min(3, ct - g3)
                            ps3 = pst.tile([128, 390], F32, tag="tab", name="l0ps")
                            for j in range(bn):
                                nc.tensor.matmul(ps3[:, j * 130:j * 130 + 129],
                                                 xgt[:, g3 + j, :],
                                                 waug[:, 0, 0:129], start=True, stop=True,
                                                 skip_group_check=True)
                            ps3v = ps3[:].rearrange("p (t e) -> p t e", e=130)
                            nc.scalar.activation(gt16[:, g3:g3 + bn, 0:128],
                                                 ps3v[:, 0:bn, 0:128], AF.Copy)
                            nc.vector.tensor_copy(as32[:, g3:g3 + bn].unsqueeze(2),
                                                  ps3v[:, 0:bn, 128:129])
                    else:
                        gt8 = gathp.tile([128, CT_MAX, ROW_B], F8, tag="g", name="gt8")
                        htab = htabs[layer]
                        tt = 0
                        si = 0
                        for (w, want_hi, nt) in meta:
                            n_seg = nt * 128
                            src_ap = htab[LO_ROWS:NPAD, :] if want_hi else htab[0:LO_ROWS, :]
                            if want_hi:
                                idxs = ihi[:, off_hi // 16:(off_hi + n_seg) // 16]
                                off_hi += n_seg
                            else:
                                idxs = ilo[:, off_lo // 16:(off_lo + n_seg) // 16]
                                off_lo += n_seg
                            nc.gpsimd.dma_gather(
                                out_ap=gt8[:, tt:tt + nt, :], in_ap=src_ap,
                                idxs_ap=idxs, num_idxs=n_seg, num_idxs_reg=n_seg,
                                elem_size=ROW_B, single_packet=False,
                                queue_num=si % NQ)
                            tt += nt
                            si += 1

                    m0t = m0tp.tile([128, CT_MAX * 128], F8, tag="m0t")
                    nc.sync.dma_start(out=m0t[:, 0:ct * 128],
                                      in_=m0t_in[:, t0 * 128:(t0 + ct) * 128])
                    m0e = m0p.tile([128, CT_MAX * 128], F8, tag="m0")
                    nc.sync.dma_start(out=m0e[:, 0:ct * 128],
                                      in_=m0e_in[:, t0 * 128:(t0 + ct) * 128])
                    adx = psadx.tile([128, 512], F32, tag="adx", name=f"adx_{layer}_{ch}")
                    for _tt in range(ct):
                        nc.tensor.matmul(adx[:, _tt:_tt + 1],
                                         m0t[:, _tt * 128:(_tt + 1) * 128],
                                         adl16[:, ftw[_tt]:ftw[_tt] + 1],
                                         start=True, stop=True, skip_group_check=True)
                    adxs = ewp.tile([128, CT_MAX], F32, tag="adxs", name="adxs")
                    nc.scalar.activation(adxs[:, 0:ct], adx[:, 0:ct], AF.Copy)

                    z  = ewp.tile([128, CT_MAX], F32, tag="z")
                    e1 = ewp.tile([128, CT_MAX], F32, tag="e1")
                    ef = ewp.tile([128, CT_MAX], F32, tag="ef")
                    if layer == 0:
                        nc.vector.tensor_tensor(z[:, 0:ct], as32[:, 0:ct], adxs[:, 0:ct], OP.add)
                    else:
                        g32 = gt8[:].bitcast(F32)
                        nc.vector.tensor_tensor(z[:, 0:ct].unsqueeze(2),
                                                g32[:, 0:ct, 32:33],
                                                adxs[:, 0:ct].unsqueeze(2), OP.add)
                    nc.scalar.activation(e1[:, 0:ct], z[:, 0:ct], AF.Exp, bias=negshift[:])
                    nc.scalar.activation(z[:, 0:ct], z[:, 0:ct], AF.Exp, bias=negshift[:], scale=0.2)
                    nc.vector.tensor_tensor(ef[:, 0:ct], e1[:, 0:ct], z[:, 0:ct], OP.max)

                    # weighted rows: gt16[:, :, 0:128] = h * ef, col 128 = ef
                    src = gt16 if layer == 0 else gt8
                    nc.vector.tensor_tensor(
                        gt16[:, 0:ct, 0:128], src[:, 0:ct, 0:128],
                        ef[:, 0:ct].unsqueeze(2).to_broadcast((128, ct, 128)), OP.mult)
                    nc.scalar.activation(gt16[:, 0:ct, 128:129],
                                         ef[:, 0:ct].unsqueeze(2), AF.Copy)

                    ft, lt = {}, {}
                    tt = 0
                    for (w, hi, nt) in meta:
                        for _ in range(nt):
                            if w not in ft: ft[w] = tt
                            lt[w] = tt
                            tt += 1
                    psws = {w: psw.tile([128, 129], F32, tag="win", name=f"win_{layer}_{ch}_{w}") for w in ft}
                    tt = 0
                    for (w, hi, nt) in meta:
                        for _ in range(nt):
                            nc.tensor.matmul(psws[w][:], m0e[:, tt * 128:(tt + 1) * 128],
                                             gt16[:, tt, 0:129],
                                             start=(tt == ft[w]), stop=(tt == lt[w]),
                                             skip_group_check=True)
                            tt += 1
                    for w in sorted(ft):
                        ps = psws[w]
                        dn = evp.tile([128, 1], F32, tag="dn")
                        nc.scalar.activation(dn[:], ps[:, 128:129], AF.Copy, bias=1e-6)
                        rc = evp.tile([128, 1], F32, tag="rc")
                        nc.vector.reciprocal(rc[:], dn[:])
                        xw = evp.tile([128, 129], F16, tag="xw")
                        nc.scalar.activation(xw[:, 0:128], ps[:, 0:128], AF.Copy, scale=rc[:])
                        nc.vector.tensor_tensor(xw[:, 0:128], xw[:, 0:128], brep[:, layer, :], OP.add)
                        nc.scalar.activation(xw[:, 0:128], xw[:, 0:128], AF.Relu)
                        if not (with_pool and last):
                            tp = psw.tile([128, 128], F16, tag="win", name=f"tp_{layer}_{ch}_{w}")
                            nc.tensor.transpose(tp[:], xw[:, 0:128], idm[:])
                            nc.vector.tensor_copy(xsT[:, w * 128:(w + 1) * 128], tp[:])
                        else:
                            nc.vector.memset(xw[:, 128:129], 1.0)
                            pps = pool_A if w < HALF_W else pool_B
                            nc.tensor.matmul(pps[:], obt[:, w, :], xw[:, 0:129],
                                             start=(w in (0, HALF_W)),
                                             stop=(w in (HALF_W - 1, WPC - 1)),
                                             skip_group_check=True)

                    if next_gen is not None and ch >= 17:
                        for _ in range(9):
                            args = next(next_gen, None)
                            if args is None:
                                break
                            emit_table_batch(*args)
                    if ch == HALF_CH:
                        if feeds_next:
                            nc.sync.dma_start(out=bi1[layer + 1][:], in_=xsT[:, 0:W_HALF0])
                            nc.gpsimd.collective_compute(
                                "AllGather", OP.bypass, replica_groups=[list(range(NCORES))],
                                ins=[bi1[layer + 1][:].opt()], outs=[bo1[layer + 1][:].opt()])
                        if with_pool and last:
                            nc.scalar.activation(pooledA[:], pool_A[:], AF.Copy)
                            nc.sync.dma_start(out=pool_biA[:], in_=pooledA[:])
                            nc.gpsimd.collective_compute(
                                "AllReduce", OP.add, replica_groups=[list(range(NCORES))],
                                ins=[pool_biA[:].opt()], outs=[pool_boA[:].opt()])
                    t0 += ct

                if feeds_next:
                    nc.sync.dma_start(out=bi2[layer + 1][:], in_=xsT[:, W_HALF0:SHARD_PAD])
                    nc.gpsimd.collective_compute(
                        "AllGather", OP.bypass, replica_groups=[list(range(NCORES))],
                        ins=[bi2[layer + 1][:].opt()], outs=[bo2[layer + 1][:].opt()])
                return pool_B

            # ================= main =================
            pool_B = None
            for layer in range(n_layers):
                gen = table_gen(layer + 1) if layer + 1 < n_layers else None
                pb = edge_phase(layer, next_gen=gen)
                if pb is not None:
                    pool_B = pb
                if gen is not None:
                    for args in gen:
                        emit_table_batch(*args)
                if layer + 1 < n_layers:
                    table_rest(layer + 1)

            if dump_xsT:
                nc.sync.dma_start(out=xsT_out[:], in_=xsT[:])
            if not with_pool:
                zz = evp.tile([128, C_OUT], F32, tag="res")
                nc.vector.memset(zz[:], 0.0)
                nc.sync.dma_start(out=out_t[:], in_=zz[:])
                return nc

            pooledB = evp.tile([128, 129], F32, tag="pooled", name="pooledB")
            nc.scalar.activation(pooledB[:], pool_B[:], AF.Copy)
            nc.sync.dma_start(out=pool_biB[:], in_=pooledB[:])
            nc.gpsimd.collective_compute(
                "AllReduce", OP.add, replica_groups=[list(range(NCORES))],
                ins=[pool_biB[:].opt()], outs=[pool_boB[:].opt()])
            pA = evp.tile([128, 129], F32, tag="pooled", name="pA")
            pB = evp.tile([128, 129], F32, tag="pooled", name="pB")
            nc.sync.dma_start(out=pA[:], in_=pool_boA[:])
            nc.sync.dma_start(out=pB[:], in_=pool_boB[:])
            pooled = evp.tile([128, 129], F32, tag="pooled", name="pooled")
            nc.vector.tensor_tensor(pooled[:], pA[:], pB[:], OP.add)
            cnt = evp.tile([128, 1], F32, tag="cnt")
            nc.vector.tensor_scalar_max(cnt[:], pooled[:, 128:129], 1.0)
            rcn = evp.tile([128, 1], F32, tag="rcn")
            nc.vector.reciprocal(rcn[:], cnt[:])
            pm = evp.tile([128, 128], F16, tag="pm")
            nc.scalar.activation(pm[:], pooled[:, 0:128], AF.Copy, scale=rcn[:])
            pt = psw.tile([128, 128], F16, tag="win", name="pt_fin")
            nc.tensor.transpose(pt[:], pm[:], idm[:])
            pts = evp.tile([128, 128], F16, tag="pts")
            nc.vector.tensor_copy(pts[:], pt[:])
            ho = psw.tile([128, 129], F32, tag="win", name="ho_fin")
            nc.tensor.matmul(ho[:, 0:C_OUT], pts[:], linw[:], start=True, stop=True,
                             skip_group_check=True)
            res = evp.tile([128, C_OUT], F32, tag="res")
            nc.vector.tensor_tensor(res[:], ho[:, 0:C_OUT], linb[:], OP.add)
            nc.sync.dma_start(out=out_t[:], in_=res[:])
    return nc


def run(inputs, trace=False, n_layers=3, with_pool=True, dump_xsT=False):
    x = np.asarray(inputs["x"])
    chunks_meta, cores, T_total, n_lo, n_hi = prep_edges(np.asarray(inputs["edge_index"]))
    const_ins = make_weight_inputs(
        np.asarray(inputs["W1"]), np.asarray(inputs["a_src1"]), np.asarray(inputs["a_dst1"]), np.asarray(inputs["b1"]),
        np.asarray(inputs["W2"]), np.asarray(inputs["a_src2"]), np.asarray(inputs["a_dst2"]), np.asarray(inputs["b2"]),
        np.asarray(inputs["W3"]), np.asarray(inputs["a_src3"]), np.asarray(inputs["a_dst3"]), np.asarray(inputs["b3"]),
        np.asarray(inputs["lin_W"]), np.asarray(inputs["lin_b"]))
    batch = np.asarray(inputs["batch"])

    nc = bacc.Bacc("TRN2", target_bir_lowering=False, debug=False,
                   num_devices=NCORES, num_swdge_queues=NQ)
    build(nc, chunks_meta, T_total, n_lo, n_hi, n_layers=n_layers,
          with_pool=with_pool, dump_xsT=dump_xsT)
    nc.compile()
    split_waits(nc)

    xf16 = x.astype(np.float16)
    in_maps = []
    for c in range(NCORES):
        m = dict(const_ins)
        m["batchl"] = make_batch_input(batch, c)
        m["obt"] = make_ob_input(m["batchl"])
        m["idxlo"] = cores[c]["idxlo"]
        m["idxhi"] = cores[c]["idxhi"]
        m0e, m0t = make_onehots(cores[c]["dstl"])
        m["m0e"] = m0e
        m["m0t"] = m0t
        m["xsT0"] = make_xsT0(x, c)
        m["xgT"] = np.ascontiguousarray(xf16[cores[c]["src_nodes"]].T)
        in_maps.append(m)
    res = bass_utils.run_bass_kernel_spmd(nc, in_maps, core_ids=list(range(NCORES)),
                                          trace=trace)
    return res.results[0], res


def kernel(**inputs):
    """Harness entry: full unsharded inputs -> [128, 10] fp32 output."""
    out, _ = run(inputs)
    if isinstance(out, dict):
        out = out["out"]
    return np.asarray(out, dtype=np.float32)


# revision 28
# speedup vs baseline: 1.0978x; 1.0171x over previous
"""GAT 3-layer Bass kernel for 8 trn2 cores.

v3: fp8 256B htab rows; per-seg gathers spread over 4 SWDGE queues
(Q7 core-pair concurrency); host-precomputed edge-major one-hot (no
on-chip is_equal); PSUM reads on the scalar engine; batched PSUM
evacuation; layer-1 gather eliminated via host-pregathered xgT; split
collectives.
"""
import numpy as np
import concourse.bacc as bacc
import concourse.bass as bass
from concourse import bass_utils
from concourse.tile import TileContext
import concourse.mybir as mybir

N, H, C_OUT, G = 50000, 128, 10, 128
NCORES = 8
NPC = N // NCORES            # 6250
WPC = 49                     # 128-node dst windows per core
CHUNK_W = 2
NCHUNK = (WPC + CHUNK_W - 1) // CHUNK_W   # 25
SHARD_PAD = WPC * 128        # 6272
NPAD = SHARD_PAD * NCORES    # 50176
ROW_B = 256                  # 256B row: [h fp8 x128 | as f32 | pad]
LO_ROWS = 32768
EXP_SHIFT = 4.0
NQ = 4                       # SWDGE queues (Q7 core pairs)
HALF_W = 25                  # windows in first collective half
HALF_CH = 12                 # after this chunk's evac, windows 0..25 are done

F16, F32, I16 = mybir.dt.float16, mybir.dt.float32, mybir.dt.int16
F8 = mybir.dt.float8e4
AF = mybir.ActivationFunctionType
OP = mybir.AluOpType


def prep_edges(edge_index):
    src = np.concatenate([edge_index[0], np.arange(N)]).astype(np.int64)
    dst = np.concatenate([edge_index[1], np.arange(N)]).astype(np.int64)
    row_id = (src // NPC) * SHARD_PAD + (src % NPC)

    per_core = []
    for c in range(NCORES):
        m = (dst // NPC) == c
        s_r, d_c = row_id[m], dst[m] - c * NPC
        win = d_c // 128
        core_chunks = []
        for ch in range(NCHUNK):
            wids = [w for w in (2 * ch, 2 * ch + 1) if w < WPC]
            segs = {}
            for hi in (0, 1):
                for w in wids:
                    mm = (win == w) & ((s_r >= LO_ROWS) == bool(hi))
                    rows = s_r[mm]
                    o = np.argsort(rows, kind="stable")
                    segs[(w, hi)] = (rows[o], (d_c[mm] - w * 128)[o])
            core_chunks.append(segs)
        per_core.append(core_chunks)

    chunks_meta = []
    for ch in range(NCHUNK):
        meta = []
        for key in per_core[0][ch]:
            w, hi = key
            mx = max(len(per_core[c][ch][key][0]) for c in range(NCORES))
            meta.append((w, hi, max(1, -(-mx // 128))))
        meta.sort(key=lambda x: (x[1], x[0]))  # lo segs first, then hi
        chunks_meta.append(meta)

    idx_lo = [[] for _ in range(NCORES)]
    idx_hi = [[] for _ in range(NCORES)]
    dstl = [[] for _ in range(NCORES)]
    rows_all = [[] for _ in range(NCORES)]
    for ch in range(NCHUNK):
        for (w, hi, ntile) in chunks_meta[ch]:
            L = ntile * 128
            for c in range(NCORES):
                rows, dl = per_core[c][ch][(w, hi)]
                r = np.zeros(L, np.int64)
                d = np.full(L, -1.0, np.float32)
                r[: len(rows)] = rows - (LO_ROWS if hi else 0)
                d[: len(dl)] = dl
                (idx_hi if hi else idx_lo)[c].append(r)
                dstl[c].append(d)
                rfull = np.zeros(L, np.int64)
                rfull[: len(rows)] = rows
                rows_all[c].append(rfull)

    def wrap16(a):
        a = a.astype(np.int16).reshape(-1, 16).T
        return np.tile(a, (8, 1)).copy()

    cores = []
    for c in range(NCORES):
        lo = np.concatenate(idx_lo[c]); hi = np.concatenate(idx_hi[c])
        dl = np.concatenate(dstl[c])
        dstl_pt = dl.reshape(-1, 128).T.copy()  # [128(edge), T]
        rows_c = np.concatenate(rows_all[c])
        shard = rows_c // SHARD_PAD
        local = rows_c % SHARD_PAD
        nodes = np.minimum(shard * NPC + local, N - 1)
        cores.append(dict(
            idxlo=wrap16(lo), idxhi=wrap16(hi),
            dstl=dstl_pt, src_nodes=nodes,
        ))
    T_total = sum(nt for ch in chunks_meta for (_, _, nt) in ch)
    n_lo = sum(nt * 128 for ch in chunks_meta for (_, hi, nt) in ch if not hi)
    n_hi = sum(nt * 128 for ch in chunks_meta for (_, hi, nt) in ch if hi)
    return chunks_meta, cores, T_total, n_lo, n_hi


def make_onehots(dstl_pt):
    """m0e[p, t*128+j] = 1 if dstl[p,t]==j (edge-major, scatter lhsT).
    m0t[n, t*128+j] = 1 if dstl[j,t]==n (dst-major, adx lhsT)."""
    T = dstl_pt.shape[1]
    ar = np.arange(128)
    m0e = (dstl_pt[:, :, None] == ar[None, None, :]).reshape(128, T * 128)
    dmat = dstl_pt.T.reshape(T, 128)
    m0t = (ar[:, None, None] == dmat[None, :, :]).reshape(128, T * 128)
    f8 = mybir.dt.np(F8)
    return m0e.astype(f8), m0t.astype(f8)


def make_weight_inputs(W1, a_src1, a_dst1, b1, W2, a_src2, a_dst2, b2,
                       W3, a_src3, a_dst3, b3, lin_W, lin_b):
    waug = np.zeros((128, 3, 130), np.float16)
    brep = np.zeros((128, 3, 128), np.float16)
    for i, (W, asr, ads, b) in enumerate([(W1, a_src1, a_dst1, b1),
                                          (W2, a_src2, a_dst2, b2),
                                          (W3, a_src3, a_dst3, b3)]):
        waug[:, i, 0:128] = W.astype(np.float32)
        waug[:, i, 128] = (W.astype(np.float64) @ asr.astype(np.float64)).astype(np.float32)
        waug[:, i, 129] = (W.astype(np.float64) @ ads.astype(np.float64)).astype(np.float32)
        brep[:, i, :] = np.broadcast_to(b.astype(np.float32), (128, 128))
    iota = np.broadcast_to(np.arange(128, dtype=np.float16), (128, 128)).copy()
    return dict(
        waug=waug, brep=brep,
        linw=lin_W.astype(np.float16),
        linb=np.broadcast_to(lin_b.astype(np.float32), (128, C_OUT)).copy(),
        iota=iota, idm=np.eye(128, dtype=np.float16),
    )


def make_xsT0(x, core):
    out = np.zeros((128, SHARD_PAD), np.float16)
    out[:, :NPC] = x[core * NPC:(core + 1) * NPC].astype(np.float16).T
    return out


def make_batch_input(batch, core):
    bl = np.full((128, WPC), -1.0, np.float32)
    ids = batch[core * NPC:(core + 1) * NPC].astype(np.float32)
    for w in range(WPC):
        seg = ids[w * 128:(w + 1) * 128]
        bl[: len(seg), w] = seg
    return bl


def make_ob_input(bl):
    """obt[p, w, j] = 1 if batch[node (w,p)] == j  (pool one-hot)."""
    return (bl[:, :, None] == np.arange(128)[None, None, :]).astype(np.float16)


def split_waits(nc, maxw=1):
    n = 0
    for func in nc.m.functions:
        for block in func.blocks:
            new = []
            for inst in block.instructions:
                si = inst.sync_info
                if si is not None and si.on_wait and len(si.on_wait) > maxw:
                    w = list(si.on_wait); extra, keep = w[:-maxw], w[-maxw:]
                    while extra:
                        ck, extra = extra[:maxw], extra[maxw:]
                        new.append(mybir.InstNoOp(name=f"ws-{n}", engine=inst.engine,
                            sync_info=mybir.SyncInfo(on_wait=ck, on_update=[])))
                        n += 1
                    si.on_wait = keep
                new.append(inst)
            block.instructions = new
    return n


def build(nc, chunks_meta, T_total, n_lo, n_hi, n_layers=3, with_pool=True,
          dump_xsT=False):
    waug_in = nc.dram_tensor("waug", [128, 3, 130], F16, kind="ExternalInput")
    brep_in = nc.dram_tensor("brep", [128, 3, 128], F16, kind="ExternalInput")
    linw_in = nc.dram_tensor("linw", [128, C_OUT], F16, kind="ExternalInput")
    linb_in = nc.dram_tensor("linb", [128, C_OUT], F32, kind="ExternalInput")
    iota_in = nc.dram_tensor("iota", [128, 128], F16, kind="ExternalInput")
    idm_in  = nc.dram_tensor("idm", [128, 128], F16, kind="ExternalInput")
    bl_in   = nc.dram_tensor("batchl", [128, WPC], F32, kind="ExternalInput")
    obt_in  = nc.dram_tensor("obt", [128, WPC, 128], F16, kind="ExternalInput")
    ilo_in  = nc.dram_tensor("idxlo", [128, n_lo // 16], I16, kind="ExternalInput")
    ihi_in  = nc.dram_tensor("idxhi", [128, n_hi // 16], I16, kind="ExternalInput")
    m0e_in  = nc.dram_tensor("m0e", [128, T_total * 128], F8, kind="ExternalInput")
    m0t_in  = nc.dram_tensor("m0t", [128, T_total * 128], F8, kind="ExternalInput")
    xsT0_in = nc.dram_tensor("xsT0", [128, SHARD_PAD], F16, kind="ExternalInput")
    xgT_in  = nc.dram_tensor("xgT", [128, T_total * 128], F16, kind="ExternalInput")
    out_t   = nc.dram_tensor("out", [G, C_OUT], F32, kind="ExternalOutput")
    xsT_out = nc.dram_tensor("xsT_out", [128, SHARD_PAD], F16, kind="ExternalOutput") if dump_xsT else None

    CT_MAX = max(sum(nt for (_, _, nt) in ch) for ch in chunks_meta)
    W_HALF0 = HALF_W * 128
    W_HALF1 = SHARD_PAD - W_HALF0

    with TileContext(nc) as tc:
        with tc.tile_pool(name="const", bufs=1) as constp, \
             tc.tile_pool(name="xTp", bufs=1) as xtp, \
             tc.tile_pool(name="gath", bufs=4) as gathp, \
             tc.tile_pool(name="gt16p", bufs=3) as gt16p, \
             tc.tile_pool(name="m0p", bufs=3) as m0p, \
             tc.tile_pool(name="ewp", bufs=3) as ewp, \
             tc.tile_pool(name="evac", bufs=3) as evp, \
             tc.tile_pool(name="stage", bufs=3) as stp, \
             tc.tile_pool(name="psw", bufs=3, space="PSUM") as psw, \
             tc.tile_pool(name="pst", bufs=3, space="PSUM") as pst, \
             tc.tile_pool(name="psadx", bufs=2, space="PSUM") as psadx, \
             tc.tile_pool(name="m0tp", bufs=3) as m0tp, \
             tc.tile_pool(name="dram", bufs=1, space="DRAM") as dram:

            xsT  = xtp.tile([128, SHARD_PAD], F16)
            waug = constp.tile([128, 3, 130], F16)
            brep = constp.tile([128, 3, 128], F16)
            linw = constp.tile([128, C_OUT], F16)
            linb = constp.tile([128, C_OUT], F32)
            iota = constp.tile([128, 128], F16)
            idm  = constp.tile([128, 128], F16)
            bl   = constp.tile([128, WPC], F32)
            obt  = constp.tile([128, WPC, 128], F16)
            ilo  = constp.tile([128, n_lo // 16], I16)
            ihi  = constp.tile([128, n_hi // 16], I16)
            nc.sync.dma_start(out=xsT[:], in_=xsT0_in[:])
            for t, s in [(waug, waug_in), (brep, brep_in),
                         (linw, linw_in), (linb, linb_in), (iota, iota_in),
                         (idm, idm_in), (bl, bl_in), (obt, obt_in),
                         (ilo, ilo_in), (ihi, ihi_in)]:
                nc.sync.dma_start(out=t[:], in_=s[:])

            negshift = constp.tile([128, 1], F32)
            nc.vector.memset(negshift[:], -EXP_SHIFT)
            eps = constp.tile([128, 1], F32)
            nc.vector.memset(eps[:], 1e-6)
            htabs = {lyr: dram.tile([NPAD, ROW_B], F8, name=f"htab_{lyr}", tag=f"htab_{lyr}")
                     for lyr in range(1, n_layers)}
            bi1 = {}; bo1 = {}; bi2 = {}; bo2 = {}
            for lyr in range(1, n_layers):
                bi1[lyr] = dram.tile([128, W_HALF0], F16, name=f"bi1_{lyr}", tag=f"bi1_{lyr}")
                bo1[lyr] = dram.tile([NCORES, 128, W_HALF0], F16, addr_space="Shared", name=f"bo1_{lyr}", tag=f"bo1_{lyr}")
                bi2[lyr] = dram.tile([128, W_HALF1], F16, name=f"bi2_{lyr}", tag=f"bi2_{lyr}")
                bo2[lyr] = dram.tile([NCORES, 128, W_HALF1], F16, addr_space="Shared", name=f"bo2_{lyr}", tag=f"bo2_{lyr}")
            pool_biA = dram.tile([128, 129], F32)
            pool_boA = dram.tile([128, 129], F32, addr_space="Shared")
            pool_biB = dram.tile([128, 129], F32)
            pool_boB = dram.tile([128, 129], F32, addr_space="Shared")

            # =========================================================
            B3 = 3

            def emit_table_batch(layer, bo, col0, r, w0, bn):
                """One 3-window batch of the fp8 htab rebuild for `layer`."""
                sti = stp.tile([128, B3 * 128], F16, tag="sti", name="sti")
                nc.sync.dma_start(
                    out=sti[:, 0:bn * 128],
                    in_=bo[r, :, w0 * 128:(w0 + bn) * 128])
                ps3 = pst.tile([128, 390], F32, tag="tab", name="tabps")
                for j in range(bn):
                    nc.tensor.matmul(ps3[:, j * 130:j * 130 + 129],
                                     sti[:, j * 128:(j + 1) * 128],
                                     waug[:, layer, 0:129], start=True, stop=True,
                                     skip_group_check=True)
                stg = stp.tile([128, B3, ROW_B], F8, tag="stg", name="stg")
                stg32 = stg[:].bitcast(F32)
                ps3v = ps3[:].rearrange("p (t e) -> p t e", e=130)
                nc.scalar.activation(stg[:, 0:bn, 0:128], ps3v[:, 0:bn, 0:128], AF.Copy)
                nc.vector.tensor_copy(stg32[:, 0:bn, 32:33], ps3v[:, 0:bn, 128:129])
                row0 = r * SHARD_PAD + (col0 + w0) * 128
                nc.sync.dma_start(
                    out=htabs[layer][row0:row0 + bn * 128, :]
                        .rearrange("(b p) e -> p b e", p=128),
                    in_=stg[:, 0:bn, :])

            def table_gen(layer):
                """Half-1 batches (windows 0..HALF_W-1) — interleavable."""
                for r in range(NCORES):
                    for w0 in range(0, HALF_W, B3):
                        yield (layer, bo1[layer], 0, r, w0, min(B3, HALF_W - w0))

            def table_rest(layer):
                """Half-2 batches (windows HALF_W..WPC-1)."""
                nwin = WPC - HALF_W
                for r in range(NCORES):
                    for w0 in range(0, nwin, B3):
                        emit_table_batch(layer, bo2[layer], HALF_W, r, w0,
                                         min(B3, nwin - w0))

            # =========================================================
            def edge_phase(layer, next_gen=None):
                last = (layer == n_layers - 1)
                feeds_next = (layer + 1 < n_layers)
                adlps = pst.tile([128, WPC], F32, tag="tab", name=f"adlps_{layer}")
                for w in range(WPC):
                    nc.tensor.matmul(adlps[:, w:w + 1], xsT[:, w * 128:(w + 1) * 128],
                                     waug[:, layer, 129:130], start=True, stop=True,
                                     skip_group_check=True)
                adl16 = ewp.tile([128, WPC], F16, tag="adl16", name=f"adl16_{layer}", bufs=1)
                nc.scalar.activation(adl16[:], adlps[:], AF.Copy)
                pool_A = pst.tile([128, 129], F32, tag="tab", name="pool_A") if (with_pool and last) else None
                pool_B = pst.tile([128, 129], F32, tag="tab", name="pool_B") if (with_pool and last) else None
                pooledA = evp.tile([128, 129], F32, tag="pooled", name="pooledA") if (with_pool and last) else None

                t0 = 0; off_lo = 0; off_hi = 0
                for ch, meta in enumerate(chunks_meta):
                    ct = sum(nt for (_, _, nt) in meta)
                    ftw = {}
                    _tt = 0
                    for (w, hi, nt) in meta:
                        for _ in range(nt):
                            ftw[_tt] = w
                            _tt += 1

                    gt16 = gt16p.tile([128, CT_MAX, 130], F16, tag="g16")
                    as32 = ewp.tile([128, CT_MAX], F32, tag="as32", name="as32") if layer == 0 else None
                    gt8 = None
                    if layer == 0:
                        xgt = gathp.tile([128, CT_MAX, 128], F16, tag="g", name="xgt")
                        nc.sync.dma_start(out=xgt[:, 0:ct, :],
                                          in_=xgT_in[:, t0 * 128:(t0 + ct) * 128]
                                              .rearrange("p (t e) -> p t e", e=128))
                        for g3 in range(0, ct, 3):
                            bn = min(3, ct - g3)
                            ps3 = pst.tile([128, 390], F32, tag="tab", name="l0ps")
                            for j in range(bn):
                                nc.tensor.matmul(ps3[:, j * 130:j * 130 + 129],
                                                 xgt[:, g3 + j, :],
                                                 waug[:, 0, 0:129], start=True, stop=True,
                                                 skip_group_check=True)
                            ps3v = ps3[:].rearrange("p (t e) -> p t e", e=130)
                            nc.scalar.activation(gt16[:, g3:g3 + bn, 0:128],
                                                 ps3v[:, 0:bn, 0:128], AF.Copy)
                            nc.vector.tensor_copy(as32[:, g3:g3 + bn].unsqueeze(2),
                                                  ps3v[:, 0:bn, 128:129])
                    else:
                        gt8 = gathp.tile([128, CT_MAX, ROW_B], F8, tag="g", name="gt8")
                        htab = htabs[layer]
                        tt = 0
                        si = 0
                        for (w, want_hi, nt) in meta:
                            n_seg = nt * 128
                            src_ap = htab[LO_ROWS:NPAD, :] if want_hi else htab[0:LO_ROWS, :]
                            if want_hi:
                                idxs = ihi[:, off_hi // 16:(off_hi + n_seg) // 16]
                                off_hi += n_seg
                            else:
                                idxs = ilo[:, off_lo // 16:(off_lo + n_seg) // 16]
                                off_lo += n_seg
                            nc.gpsimd.dma_gather(
                                out_ap=gt8[:, tt:tt + nt, :], in_ap=src_ap,
                                idxs_ap=idxs, num_idxs=n_seg, num_idxs_reg=n_seg,
                                elem_size=ROW_B, single_packet=False,
                                queue_num=si % NQ)
                            tt += nt
                            si += 1

                    m0t = m0tp.tile([128, CT_MAX * 128], F8, tag="m0t")
                    nc.sync.dma_start(out=m0t[:, 0:ct * 128],
                                      in_=m0t_in[:, t0 * 128:(t0 + ct) * 128])
                    m0e = m0p.tile([128, CT_MAX * 128], F8, tag="m0")
                    nc.sync.dma_start(out=m0e[:, 0:ct * 128],
                                      in_=m0e_in[:, t0 * 128:(t0 + ct) * 128])
                    adx = psadx.tile([128, 512], F32, tag="adx", name=f"adx_{layer}_{ch}")
                    for _tt in range(ct):
                        nc.tensor.matmul(adx[:, _tt:_tt + 1],
                                         m0t[:, _tt * 128:(_tt + 1) * 128],
                                         adl16[:, ftw[_tt]:ftw[_tt] + 1],
                                         start=True, stop=True, skip_group_check=True)
                    adxs = ewp.tile([128, CT_MAX], F32, tag="adxs", name="adxs")
                    nc.scalar.activation(adxs[:, 0:ct], adx[:, 0:ct], AF.Copy)

                    z  = ewp.tile([128, CT_MAX], F32, tag="z")
                    e1 = ewp.tile([128, CT_MAX], F32, tag="e1")
                    ef = ewp.tile([128, CT_MAX], F32, tag="ef")
                    if layer == 0:
                        nc.vector.tensor_tensor(z[:, 0:ct], as32[:, 0:ct], adxs[:, 0:ct], OP.add)
                    else:
                        g32 = gt8[:].bitcast(F32)
                        nc.vector.tensor_tensor(z[:, 0:ct].unsqueeze(2),
                                                g32[:, 0:ct, 32:33],
                                                adxs[:, 0:ct].unsqueeze(2), OP.add)
                    nc.scalar.activation(e1[:, 0:ct], z[:, 0:ct], AF.Exp, bias=negshift[:])
                    nc.scalar.activation(z[:, 0:ct], z[:, 0:ct], AF.Exp, bias=negshift[:], scale=0.2)
                    nc.vector.tensor_tensor(ef[:, 0:ct], e1[:, 0:ct], z[:, 0:ct], OP.max)

                    # weighted rows: gt16[:, :, 0:128] = h * ef, col 128 = ef
                    # (two slices so the first scatter matmuls start earlier)
                    src = gt16 if layer == 0 else gt8
                    cmid = ct // 2
                    for c0, c1 in ((0, cmid), (cmid, ct)):
                        if c1 > c0:
                            cn = c1 - c0
                            nc.vector.tensor_tensor(
                                gt16[:, c0:c1, 0:128], src[:, c0:c1, 0:128],
                                ef[:, c0:c1].unsqueeze(2).to_broadcast((128, cn, 128)),
                                OP.mult)
                    nc.scalar.activation(gt16[:, 0:ct, 128:129],
                                         ef[:, 0:ct].unsqueeze(2), AF.Copy)

                    ft, lt = {}, {}
                    tt = 0
                    for (w, hi, nt) in meta:
                        for _ in range(nt):
                            if w not in ft: ft[w] = tt
                            lt[w] = tt
                            tt += 1
                    psws = {w: psw.tile([128, 129], F32, tag="win", name=f"win_{layer}_{ch}_{w}") for w in ft}
                    tt = 0
                    for (w, hi, nt) in meta:
                        for _ in range(nt):
                            nc.tensor.matmul(psws[w][:], m0e[:, tt * 128:(tt + 1) * 128],
                                             gt16[:, tt, 0:129],
                                             start=(tt == ft[w]), stop=(tt == lt[w]),
                                             skip_group_check=True)
                            tt += 1
                    for w in sorted(ft):
                        ps = psws[w]
                        dn = evp.tile([128, 1], F32, tag="dn")
                        nc.scalar.activation(dn[:], ps[:, 128:129], AF.Copy, bias=1e-6)
                        rc = evp.tile([128, 1], F32, tag="rc")
                        nc.vector.reciprocal(rc[:], dn[:])
                        xw = evp.tile([128, 129], F16, tag="xw")
                        nc.scalar.activation(xw[:, 0:128], ps[:, 0:128], AF.Copy, scale=rc[:])
                        nc.vector.tensor_tensor(xw[:, 0:128], xw[:, 0:128], brep[:, layer, :], OP.add)
                        nc.scalar.activation(xw[:, 0:128], xw[:, 0:128], AF.Relu)
                        if not (with_pool and last):
                            tp = psw.tile([128, 128], F16, tag="win", name=f"tp_{layer}_{ch}_{w}")
                            nc.tensor.transpose(tp[:], xw[:, 0:128], idm[:])
                            nc.vector.tensor_copy(xsT[:, w * 128:(w + 1) * 128], tp[:])
                        else:
                            nc.vector.memset(xw[:, 128:129], 1.0)
                            pps = pool_A if w < HALF_W else pool_B
                            nc.tensor.matmul(pps[:], obt[:, w, :], xw[:, 0:129],
                                             start=(w in (0, HALF_W)),
                                             stop=(w in (HALF_W - 1, WPC - 1)),
                                             skip_group_check=True)

                    if next_gen is not None and ch >= 17:
                        for _ in range(9):
                            args = next(next_gen, None)
                            if args is None:
                                break
                            emit_table_batch(*args)
                    if ch == HALF_CH:
                        if feeds_next:
                            nc.sync.dma_start(out=bi1[layer + 1][:], in_=xsT[:, 0:W_HALF0])
                            nc.gpsimd.collective_compute(
                                "AllGather", OP.bypass, replica_groups=[list(range(NCORES))],
                                ins=[bi1[layer + 1][:].opt()], outs=[bo1[layer + 1][:].opt()])
                        if with_pool and last:
                            nc.scalar.activation(pooledA[:], pool_A[:], AF.Copy)
                            nc.sync.dma_start(out=pool_biA[:], in_=pooledA[:])
                            nc.gpsimd.collective_compute(
                                "AllReduce", OP.add, replica_groups=[list(range(NCORES))],
                                ins=[pool_biA[:].opt()], outs=[pool_boA[:].opt()])
                    t0 += ct

                if feeds_next:
                    nc.sync.dma_start(out=bi2[layer + 1][:], in_=xsT[:, W_HALF0:SHARD_PAD])
                    nc.gpsimd.collective_compute(
                        "AllGather", OP.bypass, replica_groups=[list(range(NCORES))],
                        ins=[bi2[layer + 1][:].opt()], outs=[bo2[layer + 1][:].opt()])
                return pool_B

            # ================= main =================
            pool_B = None
            for layer in range(n_layers):
                gen = table_gen(layer + 1) if layer + 1 < n_layers else None
                pb = edge_phase(layer, next_gen=gen)
                if pb is not None:
                    pool_B = pb
                if gen is not None:
                    for args in gen:
                        emit_table_batch(*args)
                if layer + 1 < n_layers:
                    table_rest(layer + 1)

            if dump_xsT:
                nc.sync.dma_start(out=xsT_out[:], in_=xsT[:])
            if not with_pool:
                zz = evp.tile([128, C_OUT], F32, tag="res")
                nc.vector.memset(zz[:], 0.0)
                nc.sync.dma_start(out=out_t[:], in_=zz[:])
                return nc

            pooledB = evp.tile([128, 129], F32, tag="pooled", name="pooledB")
            nc.scalar.activation(pooledB[:], pool_B[:], AF.Copy)
            nc.sync.dma_start(out=pool_biB[:], in_=pooledB[:])
            nc.gpsimd.collective_compute(
                "AllReduce", OP.add, replica_groups=[list(range(NCORES))],
                ins=[pool_biB[:].opt()], outs=[pool_boB[:].opt()])
            pA = evp.tile([128, 129], F32, tag="pooled", name="pA")
            pB = evp.tile([128, 129], F32, tag="pooled", name="pB")
            nc.sync.dma_start(out=pA[:], in_=pool_boA[:])
            nc.sync.dma_start(out=pB[:], in_=pool_boB[:])
            pooled = evp.tile([128, 129], F32, tag="pooled", name="pooled")
            nc.vector.tensor_tensor(pooled[:], pA[:], pB[:], OP.add)
            cnt = evp.tile([128, 1], F32, tag="cnt")
            nc.vector.tensor_scalar_max(cnt[:], pooled[:, 128:129], 1.0)
            rcn = evp.tile([128, 1], F32, tag="rcn")
            nc.vector.reciprocal(rcn[:], cnt[:])
            pm = evp.tile([128, 128], F16, tag="pm")
            nc.scalar.activation(pm[:], pooled[:, 0:128], AF.Copy, scale=rcn[:])
            pt = psw.tile([128, 128], F16, tag="win", name="pt_fin")
            nc.tensor.transpose(pt[:], pm[:], idm[:])
            pts = evp.tile([128, 128], F16, tag="pts")
            nc.vector.tensor_copy(pts[:], pt[:])
            ho = psw.tile([128, 129], F32, tag="win", name="ho_fin")
            nc.tensor.matmul(ho[:, 0:C_OUT], pts[:], linw[:], start=True, stop=True,
                             skip_group_check=True)
            res = evp.tile([128, C_OUT], F32, tag="res")
            nc.vector.tensor_tensor(res[:], ho[:, 0:C_OUT], linb[:], OP.add)
            nc.sync.dma_start(out=out_t[:], in_=res[:])
    return nc


def run(inputs, trace=False, n_layers=3, with_pool=True, dump_xsT=False):
    x = np.asarray(inputs["x"])
    chunks_meta, cores, T_total, n_lo, n_hi = prep_edges(np.asarray(inputs["edge_index"]))
    const_ins = make_weight_inputs(
        np.asarray(inputs["W1"]), np.asarray(inputs["a_src1"]), np.asarray(inputs["a_dst1"]), np.asarray(inputs["b1"]),
        np.asarray(inputs["W2"]), np.asarray(inputs["a_src2"]), np.asarray(inputs["a_dst2"]), np.asarray(inputs["b2"]),
        np.asarray(inputs["W3"]), np.asarray(inputs["a_src3"]), np.asarray(inputs["a_dst3"]), np.asarray(inputs["b3"]),
        np.asarray(inputs["lin_W"]), np.asarray(inputs["lin_b"]))
    batch = np.asarray(inputs["batch"])

    nc = bacc.Bacc("TRN2", target_bir_lowering=False, debug=False,
                   num_devices=NCORES, num_swdge_queues=NQ)
    build(nc, chunks_meta, T_total, n_lo, n_hi, n_layers=n_layers,
          with_pool=with_pool, dump_xsT=dump_xsT)
    nc.compile()
    split_waits(nc)

    xf16 = x.astype(np.float16)
    in_maps = []
    for c in range(NCORES):
        m = dict(const_ins)
        m["batchl"] = make_batch_input(batch, c)
        m["obt"] = make_ob_input(m["batchl"])
        m["idxlo"] = cores[c]["idxlo"]
        m["idxhi"] = cores[c]["idxhi"]
        m0e, m0t = make_onehots(cores[c]["dstl"])
        m["m0e"] = m0e
        m["m0t"] = m0t
        m["xsT0"] = make_xsT0(x, c)
        m["xgT"] = np.ascontiguousarray(xf16[cores[c]["src_nodes"]].T)
        in_maps.append(m)
    res = bass_utils.run_bass_kernel_spmd(nc, in_maps, core_ids=list(range(NCORES)),
                                          trace=trace)
    return res.results[0], res


def kernel(**inputs):
    """Harness entry: full unsharded inputs -> [128, 10] fp32 output."""
    out, _ = run(inputs)
    if isinstance(out, dict):
        out = out["out"]
    return np.asarray(out, dtype=np.float32)
